# revision 3
# baseline (speedup 1.0000x reference)
"""GraphSAGE (2-layer + edge head) distributed Bass kernel for 8 TRN2 cores.

Strategy (dst-sharded, edge-parallel):
 - Nodes partitioned contiguously: core c owns nodes [12500c, 12500(c+1)).
 - Edge e lives on the core owning dst[e]; per core, edges are grouped by
   128-dst-node "bin" and by src-table "phase" (4 slabs of 25088 rows so
   dma_gather's int16 indices fit), each (bin, phase) cell padded to a
   uniform capacity so the SPMD instruction schedule is identical on all
   cores.
 - Per layer: dma_gather fetches per-edge source rows from an AllGather'd
   node table; aggregation is one-hot matmuls accumulating into PSUM
   (onehot = is_equal(iota, dst_in_bin); pads use -1 so they vanish).
 - Algebraic folds: layer-2 pre-transforms the table by W2l (64-wide rows);
   the mean divides after the W1l matmul; biases ride ones-rows/columns;
   the edge head reduces to raw[e] = u[src]+v[dst] with u,v per-node scalars
   gathered via 4-byte dma_gather descriptors from 256B-stride tables.
"""
import numpy as np

import concourse.bass as bass
import concourse.tile as tile
import concourse.mybir as mybir
from concourse import bacc
from concourse.bass_utils import run_bass_kernel_spmd
from concourse.library_config import mlp
from concourse.masks import make_identity

F32 = mybir.dt.float32
I16 = mybir.dt.int16
P = 128

N_NODES = 100000
N_EDGES = 2000000
NCORES = 8
S = N_NODES // NCORES          # 12500 real nodes per core
NBINS = (S + P - 1) // P       # 98
SP = NBINS * P                 # 12544 padded nodes per core
TROWS = NCORES * SP            # 100352 table rows
NPHASE = 4
PHASE = TROWS // NPHASE        # 25088 (< 32768, int16-safe)

IN_X, IN_E, HID, OUT = 32, 64, 128, 64


# ----------------------------------------------------------------- host prep

def _wrap_idx(idx, cap):
    """idx [n] -> [128, cap//16] int16 (idx i at partition i%16, col i//16,
    replicated across the 8 groups of 16 partitions)."""
    t = np.zeros((16, cap // 16), np.int16)
    n = len(idx)
    t[np.arange(n) % 16, np.arange(n) // 16] = idx
    return np.tile(t, (8, 1))


def _prep(x, edge_index):
    src = np.asarray(edge_index[0], dtype=np.int64)
    dst = np.asarray(edge_index[1], dtype=np.int64)
    # global node id -> AllGather table row
    def row_of(n):
        return (n // S) * SP + (n % S)

    src_row = row_of(src)
    core_of_edge = dst // S
    dst_local = dst - core_of_edge * S

    # (core, bin, phase) cell assignment
    binn = dst_local >> 7
    phase = src_row // PHASE

    order = np.lexsort((phase, binn, core_of_edge))
    e_sorted = order
    c_s = core_of_edge[order]
    b_s = binn[order]
    q_s = phase[order]

    # cell boundaries
    cell_key = ((c_s * NBINS) + b_s) * NPHASE + q_s
    ncells = NCORES * NBINS * NPHASE
    counts = np.bincount(cell_key, minlength=ncells)
    cap = int(np.max(counts))
    cap = ((cap + 127) // 128) * 128
    assert cap <= 1024, f"cell capacity {cap} exceeds dma_gather call limit"
    chunks_per_call = cap // 128

    starts = np.zeros(ncells + 1, np.int64)
    np.cumsum(counts, out=starts[1:])
    pos_in_cell = np.arange(len(e_sorted)) - starts[cell_key]

    # per-core stream slot: call = b*4+q, slot = call*cap + pos
    call_id = b_s * NPHASE + q_s
    slot = call_id * cap + pos_in_cell
    estream = NBINS * NPHASE * cap

    gidx = np.zeros((NCORES, NBINS, NPHASE, cap), np.int16)
    vidx = np.zeros((NCORES, NBINS, NPHASE, cap), np.int16)
    dstbin = np.full((NCORES, NBINS, NPHASE, cap), -1.0, np.float32)
    gidx_f = gidx.reshape(NCORES, -1)
    vidx_f = vidx.reshape(NCORES, -1)
    dstbin_f = dstbin.reshape(NCORES, -1)
    flat = call_id * cap + pos_in_cell
    gidx_f[c_s, flat] = (src_row[order] % PHASE).astype(np.int16)
    vidx_f[c_s, flat] = (dst_local[order]).astype(np.int16)
    dstbin_f[c_s, flat] = (dst_local[order] & 127).astype(np.float32)

    # wrapped idx tiles [core, 128, NBINS, NPHASE, cap//16]
    gidx_w = np.zeros((NCORES, P, NBINS, NPHASE, cap // 16), np.int16)
    vidx_w = np.zeros((NCORES, P, NBINS, NPHASE, cap // 16), np.int16)
    for c in range(NCORES):
        for b in range(NBINS):
            for q in range(NPHASE):
                gidx_w[c, :, b, q, :] = _wrap_idx(gidx[c, b, q], cap)
                vidx_w[c, :, b, q, :] = _wrap_idx(vidx[c, b, q], cap)

    # dstbin in chunk-column layout [core, 128, NBINS*NPHASE*chunks]
    #   col = (b*NPHASE+q)*chunks + j ; partition p ; edge pos = j*128+p
    db = dstbin.reshape(NCORES, NBINS * NPHASE, chunks_per_call, P)
    dstbin_t = np.ascontiguousarray(np.transpose(db, (0, 3, 1, 2))).reshape(
        NCORES, P, NBINS * NPHASE * chunks_per_call)

    # xT slab per core [33, SP] (row 32 = ones)
    xTa = np.zeros((NCORES, IN_X + 1, SP), np.float32)
    xr = np.asarray(x, np.float32)
    for c in range(NCORES):
        xTa[c, :IN_X, :S] = xr[c * S:(c + 1) * S].T
        xTa[c, IN_X, :] = 1.0

    # output slot bookkeeping: edge e -> (core, slot)
    out_core = np.empty(len(src), np.int64)
    out_slot = np.empty(len(src), np.int64)
    out_core[order] = c_s
    out_slot[order] = slot
    return dict(cap=cap, chunks=chunks_per_call, estream=estream,
                gidx_w=gidx_w, vidx_w=vidx_w, dstbin_t=dstbin_t, xTa=xTa,
                out_core=out_core, out_slot=out_slot)


def _weights(W0, b0, W1l, b1l, W1r, W2l, b2l, W2r, Wp, bp):
    W0b = np.concatenate([W0, b0[None, :]], 0).astype(np.float32)      # [33,64]
    W1rb = np.concatenate([W1r, b1l[None, :]], 0).astype(np.float32)   # [65,128]
    Wp1 = np.asarray(Wp[:OUT, 0], np.float32)
    Wp2 = np.asarray(Wp[OUT:, 0], np.float32)
    c1 = float(np.dot(b2l, Wp1))
    c2 = float(np.dot(b2l, Wp2)) + float(bp[0])
    WpS = np.zeros((OUT + 1, 2), np.float32)
    WpS[:OUT, 0] = Wp1
    WpS[:OUT, 1] = Wp2
    WpS[OUT, 0] = c1
    WpS[OUT, 1] = c2
    return dict(W0b=W0b, W1l=np.asarray(W1l, np.float32), W1rb=W1rb,
                W2l=np.asarray(W2l, np.float32), W2r=np.asarray(W2r, np.float32),
                WpS=WpS)


# --------------------------------------------------------------- bass program

def build_program(cap, chunks, estream, nq=4):
    CPC = chunks                     # chunks per call
    NCALLS = NBINS * NPHASE
    nc = bacc.Bacc("TRN2", target_bir_lowering=False, debug=False,
                   num_swdge_queues=nq)

    xT_in = nc.declare_dram_parameter("xTa", [IN_X + 1, SP], F32, isOutput=False)
    W0b_in = nc.declare_dram_parameter("W0b", [IN_X + 1, IN_E], F32, isOutput=False)
    W1l_in = nc.declare_dram_parameter("W1l", [IN_E, HID], F32, isOutput=False)
    W1rb_in = nc.declare_dram_parameter("W1rb", [IN_E + 1, HID], F32, isOutput=False)
    W2l_in = nc.declare_dram_parameter("W2l", [HID, OUT], F32, isOutput=False)
    W2r_in = nc.declare_dram_parameter("W2r", [HID, OUT], F32, isOutput=False)
    WpS_in = nc.declare_dram_parameter("WpS", [OUT + 1, 2], F32, isOutput=False)
    iota_in = nc.declare_dram_parameter("iota", [P, P], F32, isOutput=False)
    gidx_in = nc.declare_dram_parameter("gidx", [P, NBINS, NPHASE, cap // 16], I16, isOutput=False)
    vidx_in = nc.declare_dram_parameter("vidx", [P, NBINS, NPHASE, cap // 16], I16, isOutput=False)
    dstbin_in = nc.declare_dram_parameter("dstbin", [P, NCALLS * CPC], F32, isOutput=False)
    out_ext = nc.declare_dram_parameter("out", [estream, 2], F32, isOutput=True)

    h1_shard = nc.dram_tensor("h1_shard", [SP, 128], F32)
    h1_full = nc.dram_tensor("h1_full", [TROWS, 128], F32, addr_space="Shared")
    g2_shard = nc.dram_tensor("g2_shard", [SP, OUT], F32)
    g2_full = nc.dram_tensor("g2_full", [TROWS, OUT], F32, addr_space="Shared")
    uc_shard = nc.dram_tensor("uc_shard", [SP, 1], F32)
    uc_full = nc.dram_tensor("uc_full", [TROWS, 1], F32, addr_space="Shared")
    u_pad = nc.dram_tensor("u_pad", [TROWS, 64], F32)
    v_pad = nc.dram_tensor("v_pad", [SP, 64], F32)

    RG = [list(range(NCORES))]

    def scalar_gather(eng, out_ap, table_ap, idxs_ap, nidx, queue_num):
        """dma_gather with 4-byte elems from a 256B-stride table (bypasses the
        256B-elem assert; stride_bytes_256=1)."""
        _in_ap = eng.lower_ap_dma(table_ap, for_custom_bir_dma=True)
        _idxs_ap = eng.lower_ap(idxs_ap)
        _out_ap = eng.lower_ap(out_ap)
        return eng.add_instruction(
            mybir.InstDMAGatherAnt(
                name=nc.get_next_instruction_name(),
                ins=[*_in_ap, _idxs_ap, eng.lower_val_access(eng.to_reg(nidx))],
                outs=[_out_ap],
                transpose=False, num_idxs=nidx, elem_size=1,
                stride_bytes_256=1, gen_mode=0, single_packet=True,
                queue_num=queue_num, sbuf_tokens_per_rank=0,
                sbuf_free_dim_per_rank=0, sbuf_free_dim_pad_per_rank=0,
                sbuf_byte_offset=0,
            ))

    with tile.TileContext(nc) as tc:
        nc.gpsimd.load_library(mlp)
        with tc.tile_pool(name="res", bufs=1) as res:
            ident = res.tile([P, P], F32)
            make_identity(nc, ident[:])
            iota_t = res.tile([P, P], F32)
            nc.sync.dma_start(out=iota_t[:], in_=iota_in[:])
            dstbin_t = res.tile([P, NCALLS * CPC], F32)
            nc.sync.dma_start(out=dstbin_t[:], in_=dstbin_in[:])
            W0b_t = res.tile([IN_X + 1, IN_E], F32)
            nc.sync.dma_start(out=W0b_t[:], in_=W0b_in[:])
            W1l_t = res.tile([IN_E, HID], F32)
            nc.sync.dma_start(out=W1l_t[:], in_=W1l_in[:])
            W1rb_t = res.tile([IN_E + 1, HID], F32)
            nc.sync.dma_start(out=W1rb_t[:], in_=W1rb_in[:])
            W2l_t = res.tile([HID, OUT], F32)
            nc.sync.dma_start(out=W2l_t[:], in_=W2l_in[:])
            W2r_t = res.tile([HID, OUT], F32)
            nc.sync.dma_start(out=W2r_t[:], in_=W2r_in[:])
            WpS_t = res.tile([OUT + 1, 2], F32)
            nc.sync.dma_start(out=WpS_t[:], in_=WpS_in[:])
            h1T_all = res.tile([IN_E + 1, NBINS, P], F32)
            nc.vector.memset(h1T_all[IN_E:IN_E + 1, :, :], 1.0)
            h2T_all = res.tile([HID, NBINS, P], F32)
            maxcnt_all = res.tile([P, NBINS], F32)
            uv_all = res.tile([P, NBINS, 2], F32)

            # ---------------- stage B: h1 = x @ W0 + b0 (per node chunk)
            with tc.tile_pool(name="sb_b", bufs=3) as sbb, \
                 tc.tile_pool(name="ps_b", bufs=2, space="PSUM") as psb:
                xT_t = res.tile([IN_X + 1, SP], F32)
                nc.sync.dma_start(out=xT_t[:], in_=xT_in[:])
                for b in range(NBINS):
                    psumH = psb.tile([P, IN_E], F32, tag="H")
                    nc.tensor.matmul(out=psumH[:], lhsT=xT_t[:, b * P:(b + 1) * P],
                                     rhs=W0b_t[:], start=True, stop=True)
                    h1row = sbb.tile([P, IN_E + 1], F32, tag="h1row")
                    nc.vector.memset(h1row[:, IN_E:IN_E + 1], 1.0)
                    nc.any.tensor_copy(out=h1row[:, :IN_E], in_=psumH[:])
                    nc.sync.dma_start(
                        out=h1_shard[b * P:(b + 1) * P, 0:IN_E + 1], in_=h1row[:])
                    psumT = psb.tile([IN_E, P], F32, tag="T")
                    nc.tensor.transpose(out=psumT[:], in_=h1row[:, :IN_E],
                                        identity=ident[:])
                    nc.any.tensor_copy(out=h1T_all[:IN_E, b, :], in_=psumT[:])

            tc.strict_bb_all_engine_barrier()
            nc.gpsimd.collective_compute(
                "AllGather", mybir.AluOpType.bypass, replica_groups=RG,
                ins=[h1_shard[:]], outs=[h1_full[:]])
            tc.strict_bb_all_engine_barrier()

            # ---------------- stage C: layer-1 edge pass + node update
            def edge_pass(table_full, elem, msg_cols, out_width, idx_param,
                          node_update):
                with tc.tile_pool(name="sb_e", bufs=4) as sbe, \
                     tc.tile_pool(name="ps_e", bufs=2, space="PSUM") as pse, \
                     tc.tile_pool(name="ps_n", bufs=1, space="PSUM") as psn, \
                     tc.tile_pool(name="sb_n", bufs=2) as sbn:
                    for b in range(NBINS):
                        idxb = sbe.tile([P, NPHASE, cap // 16], I16, tag="idx")
                        nc.sync.dma_start(out=idxb[:], in_=idx_param[:, b, :, :])
                        psum_b = pse.tile([P, out_width], F32, tag="agg")
                        for q in range(NPHASE):
                            msg = sbe.tile([P, CPC, elem], F32, tag="msg")
                            slab = table_full[q * PHASE:(q + 1) * PHASE, :]
                            nc.gpsimd.dma_gather(
                                msg[:], slab, idxb[:, q, :], cap, cap, elem,
                                queue_num=(b * NPHASE + q) % nq)
                            for j in range(CPC):
                                cc = (b * NPHASE + q) * CPC + j
                                onehot = sbe.tile([P, P], F32, tag="oh")
                                nc.vector.tensor_scalar(
                                    out=onehot[:], in0=iota_t[:],
                                    scalar1=dstbin_t[:, cc:cc + 1], scalar2=None,
                                    op0=mybir.AluOpType.is_equal)
                                nc.tensor.matmul(
                                    out=psum_b[:], lhsT=onehot[:],
                                    rhs=msg[:, j, :out_width],
                                    start=(q == 0 and j == 0),
                                    stop=(q == NPHASE - 1 and j == CPC - 1))
                        node_update(b, psum_b, psn, sbn)

            def l1_update(b, psum_b, psn, sbn):
                ssum = sbn.tile([P, IN_E + 1], F32, tag="ssum")
                nc.any.tensor_copy(out=ssum[:], in_=psum_b[:])
                mcx = sbn.tile([P, 1], F32, tag="mcx")
                nc.vector.tensor_scalar(
                    out=mcx[:], in0=ssum[:, IN_E:IN_E + 1],
                    scalar1=1.0, scalar2=None, op0=mybir.AluOpType.max)
                nc.vector.reciprocal(out=maxcnt_all[:, b:b + 1], in_=mcx[:])
                psumT = psn.tile([IN_E, P], F32, tag="T")
                nc.tensor.transpose(out=psumT[:], in_=ssum[:, :IN_E],
                                    identity=ident[:])
                ssumT = sbn.tile([IN_E, P], F32, tag="ssumT")
                nc.any.tensor_copy(out=ssumT[:], in_=psumT[:])
                psumA = psn.tile([P, HID], F32, tag="A")
                nc.tensor.matmul(out=psumA[:], lhsT=ssumT[:], rhs=W1l_t[:],
                                 start=True, stop=True)
                t1 = sbn.tile([P, HID], F32, tag="t1")
                nc.vector.tensor_scalar(
                    out=t1[:], in0=psumA[:], scalar1=maxcnt_all[:, b:b + 1],
                    scalar2=None, op0=mybir.AluOpType.mult)
                psumB = psn.tile([P, HID], F32, tag="B")
                nc.tensor.matmul(out=psumB[:], lhsT=h1T_all[:, b, :],
                                 rhs=W1rb_t[:], start=True, stop=True)
                t2 = sbn.tile([P, HID], F32, tag="t2")
                nc.vector.tensor_tensor(out=t2[:], in0=t1[:], in1=psumB[:],
                                        op=mybir.AluOpType.add)
                h2 = sbn.tile([P, HID], F32, tag="h2")
                nc.scalar.activation(out=h2[:], in_=t2[:],
                                     func=mybir.ActivationFunctionType.Relu)
                psumT2 = psn.tile([HID, P], F32, tag="T2")
                nc.tensor.transpose(out=psumT2[:], in_=h2[:], identity=ident[:])
                nc.any.tensor_copy(out=h2T_all[:, b, :], in_=psumT2[:])
                psumC = psn.tile([P, OUT], F32, tag="C")
                nc.tensor.matmul(out=psumC[:], lhsT=h2T_all[:, b, :],
                                 rhs=W2l_t[:], start=True, stop=True)
                g2sb = sbn.tile([P, OUT], F32, tag="g2")
                nc.any.tensor_copy(out=g2sb[:], in_=psumC[:])
                nc.sync.dma_start(out=g2_shard[b * P:(b + 1) * P, :], in_=g2sb[:])

            edge_pass(h1_full, 128, CPC, IN_E + 1, gidx_in, l1_update)

            tc.strict_bb_all_engine_barrier()
            nc.gpsimd.collective_compute(
                "AllGather", mybir.AluOpType.bypass, replica_groups=RG,
                ins=[g2_shard[:]], outs=[g2_full[:]])
            tc.strict_bb_all_engine_barrier()

            # ---------------- stage D: layer-2 edge pass + node update
            def l2_update(b, psum_b, psn, sbn):
                t3 = sbn.tile([P, OUT], F32, tag="t3")
                nc.vector.tensor_scalar(
                    out=t3[:], in0=psum_b[:], scalar1=maxcnt_all[:, b:b + 1],
                    scalar2=None, op0=mybir.AluOpType.mult)
                psumE = psn.tile([P, OUT], F32, tag="E")
                nc.tensor.matmul(out=psumE[:], lhsT=h2T_all[:, b, :],
                                 rhs=W2r_t[:], start=True, stop=True)
                h3h = sbn.tile([P, OUT], F32, tag="h3h")
                nc.vector.tensor_tensor(out=h3h[:], in0=t3[:], in1=psumE[:],
                                        op=mybir.AluOpType.add)
                psumT3 = psn.tile([OUT, P], F32, tag="T3")
                nc.tensor.transpose(out=psumT3[:], in_=h3h[:], identity=ident[:])
                uvl = sbn.tile([OUT + 1, P], F32, tag="uvl")
                nc.vector.memset(uvl[OUT:OUT + 1, :], 1.0)
                nc.any.tensor_copy(out=uvl[:OUT, :], in_=psumT3[:])
                psumUV = psn.tile([P, 2], F32, tag="UV")
                nc.tensor.matmul(out=psumUV[:], lhsT=uvl[:], rhs=WpS_t[:],
                                 start=True, stop=True)
                nc.any.tensor_copy(out=uv_all[:, b, :], in_=psumUV[:])

            edge_pass(g2_full, OUT, CPC, OUT, gidx_in, l2_update)

            # write u shard + local padded v table
            nc.sync.dma_start(
                out=uc_shard[:].rearrange("(b p) one -> p b one", p=P),
                in_=uv_all[:, :, 0:1])
            nc.sync.dma_start(
                out=v_pad[:, 0:1].rearrange("(b p) one -> p b one", p=P),
                in_=uv_all[:, :, 1:2])
            tc.strict_bb_all_engine_barrier()
            nc.gpsimd.collective_compute(
                "AllGather", mybir.AluOpType.bypass, replica_groups=RG,
                ins=[uc_shard[:]], outs=[uc_full[:]])
            tc.strict_bb_all_engine_barrier()
            # expand compact u into 256B-stride padded table (col 0)
            with nc.allow_non_contiguous_dma(reason="4B-per-row table expand"):
                for k in range(NCORES):
                    nc.sync.dma_start(
                        out=u_pad[k * SP:(k + 1) * SP, 0:1],
                        in_=uc_full[k * SP:(k + 1) * SP, :])
            tc.strict_bb_all_engine_barrier()

            # ---------------- stage E: head
            with tc.tile_pool(name="sb_h", bufs=4) as sbh:
                for b in range(NBINS):
                    gixb = sbh.tile([P, NPHASE, cap // 16], I16, tag="gix")
                    nc.sync.dma_start(out=gixb[:], in_=gidx_in[:, b, :, :])
                    vixb = sbh.tile([P, NPHASE, cap // 16], I16, tag="vix")
                    nc.sync.dma_start(out=vixb[:], in_=vidx_in[:, b, :, :])
                    for q in range(NPHASE):
                        call = b * NPHASE + q
                        ug = sbh.tile([P, CPC, 1], F32, tag="ug")
                        scalar_gather(nc.gpsimd, ug[:],
                                      u_pad[q * PHASE:(q + 1) * PHASE, 0:1],
                                      gixb[:, q, :], cap, queue_num=call % nq)
                        vg = sbh.tile([P, CPC, 1], F32, tag="vg")
                        scalar_gather(nc.gpsimd, vg[:], v_pad[:, 0:1],
                                      vixb[:, q, :], cap,
                                      queue_num=(call + 1) % nq)
                        outt = sbh.tile([P, CPC, 2], F32, tag="outt")
                        nc.vector.tensor_tensor(
                            out=outt[:, :, 0], in0=ug[:, :, 0], in1=vg[:, :, 0],
                            op=mybir.AluOpType.add)
                        nc.scalar.activation(
                            out=outt[:, :, 1], in_=outt[:, :, 0],
                            func=mybir.ActivationFunctionType.Sigmoid)
                        base = call * cap
                        nc.sync.dma_start(
                            out=out_ext[base:base + cap, :].rearrange(
                                "(j p) t -> p j t", p=P),
                            in_=outt[:])

    nc.finalize()
    return nc


_PROG_CACHE = {}


def kernel(x, edge_index, W0, b0, W1l, b1l, W1r, W2l, b2l, W2r, Wp, bp):
    prep = _prep(x, edge_index)
    wts = _weights(W0, b0, W1l, b1l, W1r, W2l, b2l, W2r, Wp, bp)
    cap, chunks, estream = prep["cap"], prep["chunks"], prep["estream"]

    key = (cap, chunks, estream)
    if key not in _PROG_CACHE:
        _PROG_CACHE[key] = build_program(cap, chunks, estream)
    nc = _PROG_CACHE[key]

    iota = np.broadcast_to(np.arange(P, dtype=np.float32), (P, P)).copy()
    in_maps = []
    for c in range(NCORES):
        in_maps.append({
            "xTa": prep["xTa"][c],
            "W0b": wts["W0b"], "W1l": wts["W1l"], "W1rb": wts["W1rb"],
            "W2l": wts["W2l"], "W2r": wts["W2r"], "WpS": wts["WpS"],
            "iota": iota,
            "gidx": np.ascontiguousarray(prep["gidx_w"][c]),
            "vidx": np.ascontiguousarray(prep["vidx_w"][c]),
            "dstbin": np.ascontiguousarray(prep["dstbin_t"][c]),
        })

    res = run_bass_kernel_spmd(nc, in_maps, core_ids=list(range(NCORES)))

    raw = np.empty(N_EDGES, np.float32)
    sig = np.empty(N_EDGES, np.float32)
    oc, os_ = prep["out_core"], prep["out_slot"]
    for c in range(NCORES):
        o = res.results[c]["out"]
        m = oc == c
        raw[m] = o[os_[m], 0]
        sig[m] = o[os_[m], 1]
    return raw, sig
